# revision 3
# baseline (speedup 1.0000x reference)
"""DeepseekV3 MoE kernel for 8x TRN2 NeuronCores.

Math: with N_ROUTED == NUM_LOCAL == 8, every top-k index is < NUM_LOCAL, so
the per-token combined routed weight is softmax(top2).sum() == 1.  The whole
module therefore reduces to

    y = down_sh(swiglu_sh(x)) + down_r(swiglu_r(x))

i.e. ONE SwiGLU MLP with concatenated intermediate dim 8192 + 1024 = 9216.

Sharding: tensor-parallel over the concatenated intermediate dim (1152 rows
per core).  Each core reads all 8192 tokens, computes a partial down-proj
output [8192, 2048]; host sums the 8 partials.

Device kernel (per core, fp32r matmuls = FP22 operands, fp32 PSUM accum):
  for each sweep of TB=1024 tokens:
    phase A: g/u = Wg/Wu-slice @ x   -> a = silu(g) * u   (a: [1152, TB])
    phase B: y[TB, 2048] (partial) = a.T @ Wd-slice       (psum-accum over i)
"""

import os
import sys

import numpy as np

for _p in ("/opt/trn_rl_repo", "/root/.axon_site/_ro/trn_rl_repo"):
    if os.path.isdir(_p):
        if _p not in sys.path:
            sys.path.insert(0, _p)
        break

from concourse import bacc, mybir, tile  # noqa: E402
from concourse.bass_utils import run_bass_kernel_spmd  # noqa: E402

N_CORES = 8
H = 2048          # hidden
I_TOT = 9216      # 8192 shared + 1024 routed intermediate
T = 8192          # tokens (4 * 2048)
IC = I_TOT // N_CORES   # 1152 intermediate rows per core
TB = 1024         # tokens per sweep

F32 = mybir.dt.float32
F32R = mybir.dt.float32r
SIGMOID = mybir.ActivationFunctionType.Sigmoid


def build_nc(h=H, ic=IC, t=T, tb=TB, n_cores=N_CORES):
    kt = h // 128    # contraction tiles for phase A
    ni = ic // 128   # intermediate 128-blocks per core
    ns = t // tb     # sweeps
    t2n = tb // 512  # 512-token chunks per sweep (phase A moving dim)
    tsn = tb // 128  # 128-token chunks per sweep (phase B output partition)
    hbn = h // 512   # 512-wide output column blocks (phase B moving dim)

    nc = bacc.Bacc("TRN2", target_bir_lowering=False, debug=False,
                   num_devices=n_cores)
    xt_d = nc.declare_dram_parameter("xt", [ns, 128, kt, tb], F32, isOutput=False)
    wg_d = nc.declare_dram_parameter("wg", [ni, 128, kt, 128], F32, isOutput=False)
    wu_d = nc.declare_dram_parameter("wu", [ni, 128, kt, 128], F32, isOutput=False)
    wd_d = nc.declare_dram_parameter("wd", [hbn, 128, ni, 512], F32, isOutput=False)
    y_d = nc.declare_dram_parameter("y", [t, h], F32, isOutput=True)

    with tile.TileContext(nc) as tc:
        with (
            tc.tile_pool(name="xp", bufs=1) as xp,
            tc.tile_pool(name="wp", bufs=4) as wp,
            tc.tile_pool(name="wdp", bufs=2) as wdp,
            tc.tile_pool(name="apool", bufs=ni) as apool,
            tc.tile_pool(name="actp", bufs=3) as actp,
            tc.tile_pool(name="yp", bufs=4) as ypool,
            tc.tile_pool(name="psA", bufs=4, space="PSUM") as psA,
            tc.tile_pool(name="psY", bufs=3, space="PSUM") as psY,
        ):
            for s in range(ns):
                xt = xp.tile([128, kt, tb], F32R, tag="xt")
                nc.sync.dma_start(xt[:], xt_d[s].bitcast(F32R))

                a_tiles = []
                for i in range(ni):
                    wg = wp.tile([128, kt, 128], F32R, tag="w")
                    nc.sync.dma_start(wg[:], wg_d[i].bitcast(F32R))
                    wu = wp.tile([128, kt, 128], F32R, tag="w")
                    nc.sync.dma_start(wu[:], wu_d[i].bitcast(F32R))
                    a_t = apool.tile([128, tb], F32R, tag="a")
                    for t2 in range(t2n):
                        tsl = slice(t2 * 512, (t2 + 1) * 512)
                        gp = psA.tile([128, 512], F32, tag="gu")
                        up = psA.tile([128, 512], F32, tag="gu")
                        for k in range(kt):
                            nc.tensor.matmul(gp[:], wg[:, k, :], xt[:, k, tsl],
                                             start=(k == 0), stop=(k == kt - 1))
                        for k in range(kt):
                            nc.tensor.matmul(up[:], wu[:, k, :], xt[:, k, tsl],
                                             start=(k == 0), stop=(k == kt - 1))
                        sl = actp.tile([128, 512], F32, tag="silu")
                        nc.scalar.activation(sl[:], gp[:], SIGMOID)
                        nc.vector.tensor_mul(sl[:], sl[:], gp[:])
                        nc.vector.tensor_mul(a_t[:, tsl], sl[:], up[:])
                    a_tiles.append(a_t)

                for hb in range(hbn):
                    wd = wdp.tile([128, ni, 512], F32R, tag="wd")
                    nc.sync.dma_start(wd[:], wd_d[hb].bitcast(F32R))
                    for ts in range(tsn):
                        yps = psY.tile([128, 512], F32, tag="y")
                        for i in range(ni):
                            nc.tensor.matmul(
                                yps[:],
                                a_tiles[i][:, ts * 128:(ts + 1) * 128],
                                wd[:, i, :],
                                start=(i == 0), stop=(i == ni - 1))
                        ysb = ypool.tile([128, 512], F32, tag="ysb")
                        nc.vector.tensor_copy(ysb[:], yps[:])
                        nc.sync.dma_start(
                            y_d[s * tb + ts * 128: s * tb + (ts + 1) * 128,
                                hb * 512:(hb + 1) * 512],
                            ysb[:])
    nc.compile()
    return nc


def prep_core_inputs(hidden_states, sh_gate, sh_up, sh_down, r_gate, r_up, r_down):
    """Host-side shard + retile.  Returns in_maps for run_bass_kernel_spmd."""
    ns, kt = T // TB, H // 128
    ni, hbn = IC // 128, H // 512

    x = np.ascontiguousarray(hidden_states, dtype=np.float32).reshape(T, H)
    # xt[s, p, k, t] = x[s*TB + t, k*128 + p]
    xt = np.ascontiguousarray(
        x.reshape(ns, TB, kt, 128).transpose(0, 3, 2, 1))

    wg_cat = np.concatenate([sh_gate, r_gate], axis=0)    # [I_TOT, H]
    wu_cat = np.concatenate([sh_up, r_up], axis=0)        # [I_TOT, H]
    wd_cat = np.concatenate([sh_down, r_down], axis=1)    # [H, I_TOT]

    in_maps = []
    for c in range(N_CORES):
        isl = slice(c * IC, (c + 1) * IC)
        # wg_t[b, p, k, i] = wg_cat[c*IC + b*128 + i, k*128 + p]
        wg_t = np.ascontiguousarray(
            wg_cat[isl].reshape(ni, 128, kt, 128).transpose(0, 3, 2, 1))
        wu_t = np.ascontiguousarray(
            wu_cat[isl].reshape(ni, 128, kt, 128).transpose(0, 3, 2, 1))
        # wd_t[hb, p, b, hcol] = wd_cat[hb*512 + hcol, c*IC + b*128 + p]
        wdslice = np.ascontiguousarray(wd_cat[:, isl].T)  # [IC, H]
        wd_t = np.ascontiguousarray(
            wdslice.reshape(ni, 128, hbn, 512).transpose(2, 1, 0, 3))
        in_maps.append({"xt": xt, "wg": wg_t, "wu": wu_t, "wd": wd_t})
    return in_maps


_NC_CACHE = {}


def _get_nc():
    if "nc" not in _NC_CACHE:
        _NC_CACHE["nc"] = build_nc()
    return _NC_CACHE["nc"]


def run(in_maps, trace=False, **kw):
    nc = _get_nc()
    return run_bass_kernel_spmd(nc, in_maps, list(range(N_CORES)),
                                trace=trace, **kw)


def kernel(hidden_states, router_weight, sh_gate, sh_up, sh_down,
           r_gate, r_up, r_down):
    in_maps = prep_core_inputs(hidden_states, sh_gate, sh_up, sh_down,
                               r_gate, r_up, r_down)
    res = run(in_maps)
    out = res.results[0]["y"].astype(np.float64)
    for c in range(1, N_CORES):
        out += res.results[c]["y"]
    return out.astype(np.float32).reshape(hidden_states.shape)


# revision 9
# speedup vs baseline: 15175.4668x; 15175.4668x over previous
"""DeepseekV3 MoE kernel for 8x TRN2 NeuronCores.

Math: with N_ROUTED == NUM_LOCAL == 8, every top-k index is < NUM_LOCAL, so
the per-token combined routed weight is softmax(top2).sum() == 1.  The whole
module therefore reduces to

    y = down_sh(swiglu_sh(x)) + down_r(swiglu_r(x))

i.e. ONE SwiGLU MLP with concatenated intermediate dim 8192 + 1024 = 9216.

Sharding: tensor-parallel over the concatenated intermediate dim (1152 rows
per core).  Each core reads all 8192 tokens, computes a partial down-proj
output [8192, 2048]; host sums the 8 partials.

Device kernel (per core, fp32r matmuls = FP22 operands, fp32 PSUM accum):
  for each sweep of TB=1024 tokens:
    phase A: g/u = Wg/Wu-slice @ x   -> a = silu(g) * u   (a: [1152, TB])
    phase B: y[TB, 2048] (partial) = a.T @ Wd-slice       (psum-accum over i)
"""

import os
import sys

import numpy as np

for _p in ("/opt/trn_rl_repo", "/root/.axon_site/_ro/trn_rl_repo"):
    if os.path.isdir(_p):
        if _p not in sys.path:
            sys.path.insert(0, _p)
        break

from concourse import bacc, mybir, tile  # noqa: E402
from concourse.bass_utils import run_bass_kernel_spmd  # noqa: E402

N_CORES = 8
H = 2048          # hidden
I_TOT = 9216      # 8192 shared + 1024 routed intermediate
T = 8192          # tokens (4 * 2048)
IC = I_TOT // N_CORES   # 1152 intermediate rows per core
TB = 1024         # tokens per sweep

F32 = mybir.dt.float32
F32R = mybir.dt.float32r
SIGMOID = mybir.ActivationFunctionType.Sigmoid


def build_nc(h=H, ic=IC, t=T, tb=TB, n_cores=N_CORES, reps=1):
    kt = h // 128    # contraction tiles for phase A
    ni = ic // 128   # intermediate 128-blocks per core
    ns = t // tb     # sweeps
    t2n = tb // 512  # 512-token chunks per sweep (phase A moving dim)
    tsn = tb // 128  # 128-token chunks per sweep (phase B output partition)
    hbn = h // 512   # 512-wide output column blocks (phase B moving dim)

    nc = bacc.Bacc("TRN2", target_bir_lowering=False, debug=False,
                   num_devices=n_cores)
    xt_d = nc.declare_dram_parameter("xt", [ns, 128, kt, tb], F32, isOutput=False)
    wg_d = nc.declare_dram_parameter("wg", [ni, 128, kt, 128], F32, isOutput=False)
    wu_d = nc.declare_dram_parameter("wu", [ni, 128, kt, 128], F32, isOutput=False)
    wd_d = nc.declare_dram_parameter("wd", [hbn, 128, ni, 512], F32, isOutput=False)
    y_d = nc.declare_dram_parameter("y", [t, h], F32, isOutput=True)

    with tile.TileContext(nc) as tc:
        import contextlib
        with (
            tc.tile_pool(name="xp", bufs=1) as xp,
            tc.tile_pool(name="wp", bufs=4) as wp,
            tc.tile_pool(name="wdp", bufs=2) as wdp,
            tc.tile_pool(name="apool", bufs=ni) as apool,
            tc.tile_pool(name="actp", bufs=3) as actp,
            tc.tile_pool(name="yp", bufs=4) as ypool,
            tc.tile_pool(name="psA", bufs=4, space="PSUM") as psA,
            tc.tile_pool(name="psY", bufs=3, space="PSUM") as psY,
            tc.For_i(0, reps, 1) if reps > 1 else contextlib.nullcontext(),
        ):
            for s in range(ns):
                xt = xp.tile([128, kt, tb], F32R, tag="xt")
                nc.sync.dma_start(xt[:], xt_d[s].bitcast(F32R))

                a_tiles = []
                for i in range(ni):
                    wg = wp.tile([128, kt, 128], F32R, tag="w")
                    nc.sync.dma_start(wg[:], wg_d[i].bitcast(F32R))
                    wu = wp.tile([128, kt, 128], F32R, tag="w")
                    nc.sync.dma_start(wu[:], wu_d[i].bitcast(F32R))
                    a_t = apool.tile([128, tb], F32R, tag="a")
                    for t2 in range(t2n):
                        tsl = slice(t2 * 512, (t2 + 1) * 512)
                        gp = psA.tile([128, 512], F32, tag="gu")
                        up = psA.tile([128, 512], F32, tag="gu")
                        for k in range(kt):
                            nc.tensor.matmul(gp[:], wg[:, k, :], xt[:, k, tsl],
                                             start=(k == 0), stop=(k == kt - 1))
                        for k in range(kt):
                            nc.tensor.matmul(up[:], wu[:, k, :], xt[:, k, tsl],
                                             start=(k == 0), stop=(k == kt - 1))
                        sl = actp.tile([128, 512], F32, tag="silu")
                        nc.scalar.activation(sl[:], gp[:], SIGMOID)
                        nc.vector.tensor_mul(sl[:], sl[:], gp[:])
                        nc.vector.tensor_mul(a_t[:, tsl], sl[:], up[:])
                    a_tiles.append(a_t)

                for hb in range(hbn):
                    wd = wdp.tile([128, ni, 512], F32R, tag="wd")
                    nc.sync.dma_start(wd[:], wd_d[hb].bitcast(F32R))
                    for ts in range(tsn):
                        yps = psY.tile([128, 512], F32, tag="y")
                        for i in range(ni):
                            nc.tensor.matmul(
                                yps[:],
                                a_tiles[i][:, ts * 128:(ts + 1) * 128],
                                wd[:, i, :],
                                start=(i == 0), stop=(i == ni - 1))
                        ysb = ypool.tile([128, 512], F32, tag="ysb")
                        nc.vector.tensor_copy(ysb[:], yps[:])
                        nc.sync.dma_start(
                            y_d[s * tb + ts * 128: s * tb + (ts + 1) * 128,
                                hb * 512:(hb + 1) * 512],
                            ysb[:])
    nc.compile()
    return nc


def prep_core_inputs(hidden_states, sh_gate, sh_up, sh_down, r_gate, r_up, r_down):
    """Host-side shard + retile.  Returns in_maps for run_bass_kernel_spmd."""
    ns, kt = T // TB, H // 128
    ni, hbn = IC // 128, H // 512

    x = np.ascontiguousarray(hidden_states, dtype=np.float32).reshape(T, H)
    # xt[s, p, k, t] = x[s*TB + t, k*128 + p]
    xt = np.ascontiguousarray(
        x.reshape(ns, TB, kt, 128).transpose(0, 3, 2, 1))

    wg_cat = np.concatenate([sh_gate, r_gate], axis=0)    # [I_TOT, H]
    wu_cat = np.concatenate([sh_up, r_up], axis=0)        # [I_TOT, H]
    wd_cat = np.concatenate([sh_down, r_down], axis=1)    # [H, I_TOT]

    in_maps = []
    for c in range(N_CORES):
        isl = slice(c * IC, (c + 1) * IC)
        # wg_t[b, p, k, i] = wg_cat[c*IC + b*128 + i, k*128 + p]
        wg_t = np.ascontiguousarray(
            wg_cat[isl].reshape(ni, 128, kt, 128).transpose(0, 3, 2, 1))
        wu_t = np.ascontiguousarray(
            wu_cat[isl].reshape(ni, 128, kt, 128).transpose(0, 3, 2, 1))
        # wd_t[hb, p, b, hcol] = wd_cat[hb*512 + hcol, c*IC + b*128 + p]
        wdslice = np.ascontiguousarray(wd_cat[:, isl].T)  # [IC, H]
        wd_t = np.ascontiguousarray(
            wdslice.reshape(ni, 128, hbn, 512).transpose(2, 1, 0, 3))
        in_maps.append({"xt": xt, "wg": wg_t, "wu": wu_t, "wd": wd_t})
    return in_maps


_NC_CACHE = {}


def _get_nc():
    if "nc" not in _NC_CACHE:
        _NC_CACHE["nc"] = build_nc()
    return _NC_CACHE["nc"]


def run(in_maps, trace=False, **kw):
    nc = _get_nc()
    return run_bass_kernel_spmd(nc, in_maps, list(range(N_CORES)),
                                trace=trace, **kw)


class Runner:
    """Persistent sharded-jit executor: stage inputs to device once, then
    time repeated kernel executions without re-trace/transfer overhead."""

    def __init__(self):
        import jax
        import mybir
        from jax.sharding import Mesh, PartitionSpec
        from jax.experimental.shard_map import shard_map
        from concourse import bass2jax

        self._jax = jax
        nc = _get_nc()
        bass2jax.install_neuronx_cc_hook()

        partition_name = (nc.partition_id_tensor.name
                          if nc.partition_id_tensor else None)
        in_names, out_names, out_avals = [], [], []
        for alloc in nc.m.functions[0].allocations:
            if not isinstance(alloc, mybir.MemoryLocationSet):
                continue
            name = alloc.memorylocations[0].name
            if alloc.kind == "ExternalInput":
                if name != partition_name:
                    in_names.append(name)
            elif alloc.kind == "ExternalOutput":
                out_names.append(name)
                out_avals.append(jax.core.ShapedArray(
                    tuple(alloc.tensor_shape), mybir.dt.np(alloc.dtype)))
        self._in_names, self._out_names, self._out_avals = \
            in_names, out_names, out_avals
        all_in = in_names + out_names
        if partition_name is not None:
            all_in = all_in + [partition_name]

        def _body(*args):
            operands = list(args)
            if partition_name is not None:
                operands.append(bass2jax.partition_id_tensor())
            outs = bass2jax._bass_exec_p.bind(
                *operands,
                out_avals=tuple(out_avals),
                in_names=tuple(all_in),
                out_names=tuple(out_names),
                lowering_input_output_aliases=(),
                sim_require_finite=True,
                sim_require_nnan=True,
                nc=nc,
            )
            return tuple(outs)

        self._body = _body
        self._chain_cache = {}
        devices = jax.devices()[:N_CORES]
        self._mesh = Mesh(np.asarray(devices), ("core",))
        n_all = len(in_names) + len(out_names)
        self._sharded = jax.jit(shard_map(
            _body, mesh=self._mesh,
            in_specs=(PartitionSpec("core"),) * n_all,
            out_specs=(PartitionSpec("core"),) * len(out_names),
            check_rep=False))
        self._dev_args = None

    def stage(self, in_maps):
        import jax
        from jax.sharding import NamedSharding, PartitionSpec

        sh = NamedSharding(self._mesh, PartitionSpec("core"))
        args = []
        for name in self._in_names:
            cat = np.concatenate([np.asarray(m[name]) for m in in_maps], axis=0)
            args.append(jax.device_put(cat, sh))
        for av in self._out_avals:
            z = np.zeros((N_CORES * av.shape[0], *av.shape[1:]), av.dtype)
            args.append(jax.device_put(z, sh))
        jax.block_until_ready(args)
        self._dev_args = args

    def execute(self):
        out = self._sharded(*self._dev_args)
        self._jax.block_until_ready(out)
        return out

    def execute_chain(self, k):
        """Run the kernel k times back-to-back inside one jit dispatch.
        Successive calls chain the output buffers, so device executions
        serialize; wall-time differences measure pure device time."""
        import jax
        from jax.experimental.shard_map import shard_map
        from jax.sharding import PartitionSpec

        if k not in self._chain_cache:
            n_in = len(self._in_names)
            n_out = len(self._out_names)
            body = self._body

            def _chain(*args):
                ins = args[:n_in]
                outs = args[n_in:]
                for _ in range(k):
                    outs = body(*ins, *outs)
                return outs

            self._chain_cache[k] = jax.jit(shard_map(
                _chain, mesh=self._mesh,
                in_specs=(PartitionSpec("core"),) * (n_in + n_out),
                out_specs=(PartitionSpec("core"),) * n_out,
                check_rep=False))
        out = self._chain_cache[k](*self._dev_args)
        self._jax.block_until_ready(out)
        return out

    def results(self, out):
        per_core = []
        for c in range(N_CORES):
            d = {}
            for i, name in enumerate(self._out_names):
                av = self._out_avals[i]
                d[name] = np.asarray(out[i]).reshape(N_CORES, *av.shape)[c]
            per_core.append(d)
        return per_core


def kernel(hidden_states, router_weight, sh_gate, sh_up, sh_down,
           r_gate, r_up, r_down):
    in_maps = prep_core_inputs(hidden_states, sh_gate, sh_up, sh_down,
                               r_gate, r_up, r_down)
    res = run(in_maps)
    out = res.results[0]["y"].astype(np.float64)
    for c in range(1, N_CORES):
        out += res.results[c]["y"]
    return out.astype(np.float32).reshape(hidden_states.shape)
